# revision 1
# baseline (speedup 1.0000x reference)
"""Trainium2 Bass kernel for nn_NeuralNetworkDPD (dense_mlp).

Strategy (feature-major, 2-token-halves packed on 128 partitions):
  - Each core handles 4 batch rows. A-half = rows {0,1}, B-half = rows {2,3},
    packed as SBUF partitions [0:64)=A-token features, [64:128)=B-token feats.
  - Dense layers: block-diag(W, W) stationary [128,128]; each streamed column
    carries 2 tokens -> 0.5 PE cycles/token/layer.
  - LayerNorm stats as broadcast PLANES: a block-diag(ones/64) stationary
    reduces over the feature partitions and replicates the result to all 64
    output partitions of each half, so mean/var arrive already broadcast:
        mu_bc  = onesd @ z      (one matmul)
        var_bc = onesd @ (z-mu_bc)^2
  - Normalize: v=(z-mu_bc); rs=recip_approx(sqrt(var_bc+eps)); u=Prelu on
    ScalarE fusing gamma (scale), beta (bias), alpha - all per-partition.
  - skip connection and b_out applied host-side (cheap rank-1/elementwise).
"""

import sys
from contextlib import ExitStack

sys.path.insert(0, "/opt/trn_rl_repo")

import numpy as np

import concourse.bacc as bacc
import concourse.bass as bass
import concourse.tile as tile
from concourse import mybir

F = 64          # feature width
NL = 6          # chained dense layers
EPS = 1e-3
CH = 512        # tokens per matmul (PSUM bank)
SUP = 8         # chunks per super-chunk (scheduling window)
R = mybir.dt.float32r   # dtype of all matmul-feeding tensors (1 cyc/row)


def build_kernel(tc, outs, ins, tokens_per_row):
    """Emit the Tile program. ins/outs are dicts of DRAM APs."""
    nc = tc.nc
    TPR = tokens_per_row
    cpr = TPR // CH              # chunks per row
    spr = cpr // SUP             # super-chunks per row
    assert cpr % SUP == 0
    NG = SUP // 2                # groups (of 2 chunks) per super

    xr, xi = ins["xr"], ins["xi"]
    out = outs["out"]            # [4, TPR, 2] fp32

    # Internal padded copies of x: [4, TPR+3], first 3 entries zero.
    xpad_r = nc.dram_tensor("xpad_r", [4, TPR + 3], R,
                            kind="Internal").ap()
    xpad_i = nc.dram_tensor("xpad_i", [4, TPR + 3], R,
                            kind="Internal").ap()

    ctx = ExitStack()
    singles = ctx.enter_context(tc.tile_pool(name="singles", bufs=1))
    zpool = ctx.enter_context(tc.tile_pool(name="zpool", bufs=8))
    rpool = ctx.enter_context(tc.tile_pool(name="rpool", bufs=8))
    upool = ctx.enter_context(tc.tile_pool(name="upool", bufs=3))
    vpool = ctx.enter_context(tc.tile_pool(name="vpool", bufs=4))
    qpool = ctx.enter_context(tc.tile_pool(name="qpool", bufs=4))
    fpool = ctx.enter_context(tc.tile_pool(name="fpool", bufs=4))
    opool = ctx.enter_context(tc.tile_pool(name="opool", bufs=3))
    zp_pool = ctx.enter_context(tc.tile_pool(name="zp", bufs=2, space="PSUM"))
    mu_pool = ctx.enter_context(tc.tile_pool(name="mu", bufs=2, space="PSUM"))
    va_pool = ctx.enter_context(tc.tile_pool(name="va", bufs=2, space="PSUM"))

    # ---- load weights/constants into SBUF ----
    wd = singles.tile([128, NL * 128], R)
    win = singles.tile([16, 128], R)
    wout = singles.tile([128, 4], R)
    onesd = singles.tile([128, 128], R)
    percol = singles.tile([128, 25], mybir.dt.float32)
    epsc = singles.tile([128, 1], mybir.dt.float32)
    nc.sync.dma_start(out=wd, in_=ins["wd"])
    nc.sync.dma_start(out=win, in_=ins["win"])
    nc.sync.dma_start(out=wout, in_=ins["wout"])
    nc.sync.dma_start(out=onesd, in_=ins["onesd"])
    nc.sync.dma_start(out=percol, in_=ins["percol"])
    nc.vector.memset(epsc, EPS)

    b_in_col = percol[:, 0:1]
    dense_b_col = [percol[:, 1 + l: 2 + l] for l in range(NL)]
    gamma_col = [percol[:, 7 + l: 8 + l] for l in range(NL)]
    beta_col = [percol[:, 13 + l: 14 + l] for l in range(NL)]
    alpha_col = [percol[:, 19 + l: 20 + l] for l in range(NL)]

    # ---- build zero-padded x in DRAM ----
    zrow = singles.tile([1, 4], R)
    nc.vector.memset(zrow.bitcast(mybir.dt.float32), 0.0)
    for r in range(4):
        for xp in (xpad_r, xpad_i):
            nc.sync.dma_start(out=xp[r: r + 1, 0:3], in_=zrow[0:1, 0:3])
    nc.sync.dma_start(out=xpad_r[:, 3:], in_=xr)
    nc.sync.dma_start(out=xpad_i[:, 3:], in_=xi)

    # ---------------- main loops ----------------
    for rp in range(2):                     # row-pair: A=row rp, B=row 2+rp
        rowA, rowB = rp, 2 + rp
        for s in range(spr):                # super-chunk
            # -- w_in: windowed feats + first dense for 8 chunks --
            zps = []                        # psum tiles holding current z
            for k in range(SUP):
                t0 = (s * SUP + k) * CH
                feats = fpool.tile([16, CH], R, tag="feats")
                # A-half lags: rows 0-3 real, 4-7 imag; B-half: rows 8-15
                for (base, row) in ((0, rowA), (8, rowB)):
                    src_r = bass.AP(tensor=xpad_r.tensor,
                                    offset=row * (TPR + 3) + t0,
                                    ap=[[1, 4], [1, CH]])
                    src_i = bass.AP(tensor=xpad_i.tensor,
                                    offset=row * (TPR + 3) + t0,
                                    ap=[[1, 4], [1, CH]])
                    nc.sync.dma_start(out=feats[base: base + 4, :], in_=src_r)
                    nc.sync.dma_start(out=feats[base + 4: base + 8, :], in_=src_i)
                if k % 2 == 0:
                    zp = zp_pool.tile([128, 2 * CH], mybir.dt.float32, tag="zp")
                    zps.append(zp)
                nc.tensor.matmul(out=zps[-1][:, (k % 2) * CH:(k % 2 + 1) * CH],
                                 lhsT=(win[:, :]), rhs=(feats),
                                 start=True, stop=True)

            res = [None, None, None]        # z0, z2, z4 anchor groups
            z_groups = [None] * NG

            for l in range(NL + 1):         # 6 LN+PReLU+dense stages + final
                bias = b_in_col if l == 0 else dense_b_col[l - 1]
                new_z = [None] * NG
                for g in range(NG):
                    if l in (0, 2, 4):
                        zt = rpool.tile([128, 2 * CH], R,
                                        tag="za", name=f"za{l}g{g}")
                    else:
                        zt = zpool.tile([128, 2 * CH], R,
                                        tag="z", name=f"z{l}g{g}")
                    nc.scalar.activation(out=zt, in_=zps[g],
                                         func=mybir.ActivationFunctionType.Identity,
                                         bias=bias, scale=1.0)
                    if l in (2, 4, 6):      # residual add at block boundaries
                        if l == 6:
                            zsum = zpool.tile([128, 2 * CH], R,
                                              tag="z", name=f"zs{l}g{g}")
                        else:
                            zsum = rpool.tile([128, 2 * CH], R,
                                              tag="zb", name=f"zs{l}g{g}")
                        nc.vector.tensor_add(zsum, zt, res[l // 2 - 1][g])
                        zt = zsum
                    new_z[g] = zt
                z_groups = new_z
                if l in (0, 2, 4):
                    res[l // 2] = z_groups
                if l == NL:
                    break

                zps = []
                for g in range(NG):
                    zg = z_groups[g]
                    u = upool.tile([128, 2 * CH], R, tag="u")
                    for j in range(2):
                        zsl = zg[:, j * CH:(j + 1) * CH]
                        # mean plane (already broadcast to both halves)
                        mu = mu_pool.tile([128, CH], mybir.dt.float32, tag="mu")
                        nc.tensor.matmul(out=mu, lhsT=(onesd[:, :]),
                                         rhs=(zsl), start=True, stop=True)
                        v = vpool.tile([128, CH], mybir.dt.float32, tag="v")
                        nc.vector.tensor_sub(v, zsl, mu)
                        vsq = qpool.tile([128, CH], R, tag="vsq")
                        nc.scalar.activation(
                            out=vsq, in_=v,
                            func=mybir.ActivationFunctionType.Square)
                        va = va_pool.tile([128, CH], mybir.dt.float32, tag="va")
                        nc.tensor.matmul(out=va, lhsT=(onesd[:, :]),
                                         rhs=(vsq), start=True, stop=True)
                        sg = qpool.tile([128, CH], mybir.dt.float32, tag="sg")
                        nc.scalar.activation(
                            out=sg, in_=va,
                            func=mybir.ActivationFunctionType.Sqrt,
                            bias=epsc, scale=1.0)
                        rs = vpool.tile([128, CH], mybir.dt.float32, tag="rs")
                        nc.vector.reciprocal_approx_fast(out=rs, in_=sg)
                        nc.vector.tensor_mul(u[:, j * CH:(j + 1) * CH], v, rs)
                    # PReLU(gamma*x + beta) fused on ScalarE, in place on u
                    nc.scalar.activation(out=u, in_=u,
                                         func=mybir.ActivationFunctionType.Prelu,
                                         bias=beta_col[l], scale=gamma_col[l],
                                         alpha=alpha_col[l])
                    zp = zp_pool.tile([128, 2 * CH], mybir.dt.float32, tag="zp")
                    for j in range(2):
                        nc.tensor.matmul(
                            out=zp[:, j * CH:(j + 1) * CH],
                            lhsT=(wd[:, l * 128:(l + 1) * 128]),
                            rhs=(u[:, j * CH:(j + 1) * CH]),
                            start=True, stop=True)
                    zps.append(zp)

            # -- w_out + store --
            for g in range(NG):
                for j in range(2):
                    k = 2 * g + j
                    t0 = (s * SUP + k) * CH
                    op = mu_pool.tile([4, CH], mybir.dt.float32, tag="mu",
                                      padded_shape=[128, CH])
                    nc.tensor.matmul(out=op, lhsT=(wout[:, :]),
                                     rhs=(z_groups[g][:, j * CH:(j + 1) * CH]),
                                     start=True, stop=True)
                    ot = opool.tile([4, CH], mybir.dt.float32, tag="ot")
                    nc.scalar.copy(out=ot, in_=op)
                    for (half, row) in ((0, rowA), (1, rowB)):
                        dst = bass.AP(tensor=out.tensor,
                                      offset=row * TPR * 2 + t0 * 2,
                                      ap=[[1, 2], [2, CH]])
                        nc.sync.dma_start(out=dst,
                                          in_=ot[2 * half: 2 * half + 2, :])
    ctx.close()


def _host_pack(inputs):
    """Build the shared (replicated) packed-weight arrays."""
    w_in = np.asarray(inputs["w_in"], np.float32)
    dense_w = np.asarray(inputs["dense_w"], np.float32)
    w_out = np.asarray(inputs["w_out"], np.float32)
    ln_gamma = np.asarray(inputs["ln_gamma"], np.float32)
    ln_beta = np.asarray(inputs["ln_beta"], np.float32)
    alpha = np.asarray(inputs["alpha"], np.float32)
    b_in = np.asarray(inputs["b_in"], np.float32)
    dense_b = np.asarray(inputs["dense_b"], np.float32)

    wd = np.zeros((128, NL * 128), np.float32)
    for l in range(NL):
        wd[0:64, l * 128: l * 128 + 64] = dense_w[l]
        wd[64:128, l * 128 + 64: l * 128 + 128] = dense_w[l]
    win = np.zeros((16, 128), np.float32)
    win[0:8, 0:64] = w_in
    win[8:16, 64:128] = w_in
    wout = np.zeros((128, 4), np.float32)
    wout[0:64, 0:2] = w_out
    wout[64:128, 2:4] = w_out
    onesd = np.zeros((128, 128), np.float32)
    onesd[0:64, 0:64] = 1.0 / F
    onesd[64:128, 64:128] = 1.0 / F
    percol = np.zeros((128, 25), np.float32)
    percol[:, 0] = np.tile(b_in, 2)
    for l in range(NL):
        percol[:, 1 + l] = np.tile(dense_b[l], 2)
        percol[:, 7 + l] = np.tile(ln_gamma[l], 2)
        percol[:, 13 + l] = np.tile(ln_beta[l], 2)
        percol[:, 19 + l] = np.tile(alpha[l], 2)
    return dict(wd=wd, win=win, wout=wout, onesd=onesd, percol=percol)


def build_program(tokens_per_row):
    """Build the full Bass/Tile program for one core's shard."""
    nc = bacc.Bacc("TRN2")
    ins = {}
    shapes = dict(wd=(128, NL * 128), win=(16, 128), wout=(128, 4),
                  onesd=(128, 128), percol=(128, 25))
    for name, shp in shapes.items():
        dt = mybir.dt.float32 if name == "percol" else R
        ins[name] = nc.dram_tensor(name, list(shp), dt,
                                   kind="ExternalInput").ap()
    ins["xr"] = nc.dram_tensor("xr", [4, tokens_per_row], R,
                               kind="ExternalInput").ap()
    ins["xi"] = nc.dram_tensor("xi", [4, tokens_per_row], R,
                               kind="ExternalInput").ap()
    outs = {"out": nc.dram_tensor("out", [4, tokens_per_row, 2],
                                  mybir.dt.float32, kind="ExternalOutput").ap()}
    with tile.TileContext(nc) as tc:
        build_kernel(tc, outs, ins, tokens_per_row)
    nc.compile()
    return nc


def _run(inputs, trace=False):
    from concourse.bass_utils import run_bass_kernel_spmd

    x_real = np.asarray(inputs["x_real"], np.float32)
    x_imag = np.asarray(inputs["x_imag"], np.float32)
    B, N = x_real.shape
    n_cores = 8
    rows_per_core = B // n_cores

    shared = _host_pack(inputs)
    nc = build_program(N)

    in_maps = []
    for c in range(n_cores):
        m = dict(shared)
        m["xr"] = np.ascontiguousarray(x_real[c * rows_per_core:(c + 1) * rows_per_core])
        m["xi"] = np.ascontiguousarray(x_imag[c * rows_per_core:(c + 1) * rows_per_core])
        in_maps.append(m)

    res = run_bass_kernel_spmd(nc, in_maps, core_ids=list(range(n_cores)),
                               trace=trace)
    outs_np = [r["out"] for r in res.results]
    full = np.concatenate(outs_np, axis=0)          # [B, N, 2]
    b_out = np.asarray(inputs["b_out"], np.float32)
    re = full[..., 0] + b_out[0] + x_real
    im = full[..., 1] + b_out[1] + x_imag
    return (re + 1j * im).astype(np.complex64), res


def kernel(**inputs):
    return _run(inputs, trace=False)[0]



# revision 2
# speedup vs baseline: 3.9445x; 3.9445x over previous
"""Trainium2 Bass kernel for nn_NeuralNetworkDPD (dense_mlp)  — v2.

Feature-major, 2-token-halves packed on 128 partitions (A rows {0,1},
B rows {2,3}; partitions [0:64)=A-token features, [64:128)=B).

v2 changes vs v1:
  - Mean-centering projection P = I - 11^T/64 folded into every weight
    matrix host-side, so each layer's activations arrive pre-centered:
    the LN mean matmul and the subtract disappear.  The lost means (the
    final  Z @ w_out  needs the uncentered Z) are reconstructed with 4
    tiny matmuls (feats*mean(w_in), u1*mean(W1), u3*mean(W3),
    u5*mean(W5)) accumulated straight into the output PSUM bank with
    colsum(w_out) folded into their weights.
  - bf16 for all SBUF-resident tensors (weights + activations): 2x DVE
    modes, FWL weight loads, faster PE streaming.  PSUM stays fp32.
  - LN variance path: Square(zp + bias) fused on ACT directly from
    PSUM -> va matmul -> Abs_reciprocal_sqrt(va + eps) on ACT (one op,
    one activation table) -> t = (zp + bias) * rs via
    scalar_tensor_tensor on DVE -> Prelu(gamma*t + beta) on ACT.
  - Residual adds fused into the PSUM->SBUF materialize via
    scalar_tensor_tensor: Z = (zp + bias) + Z_prev.
  - Planar DRAM output [4, 2, N] (contiguous stores; real/imag
    interleave done on host) - kills the v1 4-byte-descriptor storm.
  - Inputs converted fp32->bf16 on device (wide reshape + DVE copy),
    feats loaded one dma_start per half per super-chunk.
"""

import sys
from contextlib import ExitStack

sys.path.insert(0, "/opt/trn_rl_repo")

import numpy as np

import concourse.bacc as bacc
import concourse.bass as bass
import concourse.tile as tile
from concourse import mybir

F = 64          # feature width
NL = 6          # chained dense layers
EPS = 1e-3
CH = 512        # tokens per matmul (PSUM bank)
SUP = 8         # chunks per super-chunk (scheduling window)
BF = mybir.dt.bfloat16
FP = mybir.dt.float32
AF = mybir.ActivationFunctionType
OP = mybir.AluOpType

USE_ARS = True  # Abs_reciprocal_sqrt on ACT; False -> Sqrt + DVE recip


def build_kernel(tc, outs, ins, tokens_per_row):
    nc = tc.nc
    TPR = tokens_per_row
    cpr = TPR // CH
    spr = cpr // SUP
    assert cpr % SUP == 0
    NG = SUP // 2               # groups (of 2 chunks) per super
    N3 = TPR + 3

    xr, xi = ins["xr"], ins["xi"]       # [4, TPR] fp32
    out = outs["out"]                   # [4, 2, TPR] fp32 planar

    xpad = nc.dram_tensor("xpad", [4, 2, N3], BF, kind="Internal").ap()

    ctx = ExitStack()
    singles = ctx.enter_context(tc.tile_pool(name="singles", bufs=1))
    cvt32 = ctx.enter_context(tc.tile_pool(name="cvt32", bufs=2))
    cvt16 = ctx.enter_context(tc.tile_pool(name="cvt16", bufs=2))
    fpool = ctx.enter_context(tc.tile_pool(name="fpool", bufs=2))
    anchors = ctx.enter_context(tc.tile_pool(name="anchors", bufs=8))
    upool = ctx.enter_context(tc.tile_pool(name="upool", bufs=16))
    qpool = ctx.enter_context(tc.tile_pool(name="qpool", bufs=4))   # vsq
    rpool = ctx.enter_context(tc.tile_pool(name="rpool", bufs=4))   # rs
    tpool = ctx.enter_context(tc.tile_pool(name="tpool", bufs=4))   # t
    opool = ctx.enter_context(tc.tile_pool(name="opool", bufs=3))
    zp_pool = ctx.enter_context(tc.tile_pool(name="zp", bufs=2, space="PSUM"))
    sp_pool = ctx.enter_context(tc.tile_pool(name="sp", bufs=2, space="PSUM"))

    # ---- weights/constants -> SBUF ----
    wd = singles.tile([128, NL * 128], BF)        # folded dense, block-diag
    win = singles.tile([16, 128], BF)             # folded w_in, block-diag
    wout = singles.tile([128, 4], BF)             # w_out block-diag
    mw = singles.tile([128, 12], BF)              # mean-track lhsT l=1,3,5
    mwin = singles.tile([16, 4], BF)              # mean-track lhsT for feats
    onesd = singles.tile([128, 128], BF)          # block-diag ones/64
    percol = singles.tile([128, 26], FP)          # per-partition columns
    epsc = singles.tile([128, 1], FP)
    nc.sync.dma_start(out=wd, in_=ins["wd"])
    nc.sync.dma_start(out=win, in_=ins["win"])
    nc.sync.dma_start(out=wout, in_=ins["wout"])
    nc.sync.dma_start(out=mw, in_=ins["mw"])
    nc.sync.dma_start(out=mwin, in_=ins["mwin"])
    nc.sync.dma_start(out=onesd, in_=ins["onesd"])
    nc.sync.dma_start(out=percol, in_=ins["percol"])
    nc.vector.memset(epsc, EPS)

    b_in_col = percol[:, 0:1]
    bcol = [percol[:, 1 + l: 2 + l] for l in range(NL)]       # centered b_l
    gcol = [percol[:, 7 + l: 8 + l] for l in range(NL)]       # gamma
    ecol = [percol[:, 13 + l: 14 + l] for l in range(NL)]     # beta
    acol = [percol[:, 19 + l: 20 + l] for l in range(NL)]     # alpha

    # ---- convert x to bf16 into padded DRAM ----
    # [4, TPR] fp32 viewed as [128, TPR//32]: partition p = 32*row + blk
    W = TPR // 32
    zrow = singles.tile([8, 4], BF)
    nc.vector.memset(zrow, 0.0)
    dst0 = bass.AP(tensor=xpad.tensor, offset=0, ap=[[N3, 8], [1, 3]])
    nc.sync.dma_start(out=dst0, in_=zrow[:, 0:3])
    for src, h in ((xr, 0), (xi, 1)):
        ld = cvt32.tile([128, W], FP, tag="ld", name=f"ld{h}")
        sap = bass.AP(tensor=src.tensor, offset=0, ap=[[W, 128], [1, W]])
        nc.sync.dma_start(out=ld, in_=sap)
        cv = cvt16.tile([128, W], BF, tag="cv", name=f"cv{h}")
        nc.vector.tensor_copy(cv, ld)
        dap = bass.AP(tensor=xpad.tensor, offset=h * N3 + 3,
                      ap=[[2 * N3, 4], [W, 32], [1, W]])
        nc.sync.dma_start(out=dap, in_=cv)

    # ---------------- main loops ----------------
    for rp in range(2):                  # row-pair: A=row rp, B=row 2+rp
        for s in range(spr):
            t0s = s * SUP * CH
            # -- feats for the whole super-chunk: [16, SUP*CH] bf16 --
            feats = fpool.tile([16, SUP * CH], BF, tag="feats")
            for b in (0, 1):             # half: A rows / B rows
                src = bass.AP(tensor=xpad.tensor,
                              offset=(rp + 2 * b) * 2 * N3 + t0s,
                              ap=[[N3, 2], [1, 4], [1, SUP * CH]])
                nc.sync.dma_start(out=feats[b * 8:(b + 1) * 8, :], in_=src)

            def fch(k):
                return feats[:, k * CH:(k + 1) * CH]

            # -- stage 0: w_in matmuls -> a0 anchors --
            zps = []
            for g in range(NG):
                zp = zp_pool.tile([128, 2 * CH], FP, tag="zp",
                                  name=f"zp0g{g}")
                for j in range(2):
                    k = 2 * g + j
                    nc.tensor.matmul(out=zp[:, j * CH:(j + 1) * CH],
                                     lhsT=win, rhs=fch(k),
                                     start=True, stop=True)
                zps.append(zp)
            anchor = []
            for g in range(NG):
                a0 = anchors.tile([128, 2 * CH], BF, tag="anc",
                                  name=f"a0g{g}")
                nc.scalar.activation(out=a0, in_=zps[g], func=AF.Identity,
                                     bias=b_in_col, scale=1.0)
                anchor.append(a0)
            cur = list(anchor)           # LN input (SBUF) for even stages
            u_keep = [[None] * 3 for _ in range(NG)]

            for i in range(NL):          # LN stages 0..5
                even = (i % 2 == 0)
                new_zps = []
                for g in range(NG):
                    # --- variance path ---
                    vsq = qpool.tile([128, 2 * CH], BF, tag="vsq",
                                     name=f"vsq{i}g{g}")
                    if even:
                        nc.scalar.activation(out=vsq, in_=cur[g],
                                             func=AF.Square)
                    else:
                        nc.scalar.activation(out=vsq, in_=zps[g],
                                             func=AF.Square,
                                             bias=bcol[i - 1], scale=1.0)
                    va = sp_pool.tile([128, 2 * CH], FP, tag="va",
                                      name=f"va{i}g{g}")
                    for j in range(2):
                        nc.tensor.matmul(out=va[:, j * CH:(j + 1) * CH],
                                         lhsT=onesd,
                                         rhs=vsq[:, j * CH:(j + 1) * CH],
                                         start=True, stop=True)
                    rs = rpool.tile([128, 2 * CH], BF, tag="rs",
                                    name=f"rs{i}g{g}")
                    if USE_ARS:
                        nc.scalar.activation(out=rs, in_=va,
                                             func=AF.Abs_reciprocal_sqrt,
                                             bias=epsc, scale=1.0)
                    else:
                        sg = tpool.tile([128, 2 * CH], FP, tag="t",
                                        name=f"sg{i}g{g}")
                        nc.scalar.activation(out=sg, in_=va, func=AF.Sqrt,
                                             bias=epsc, scale=1.0)
                        nc.vector.reciprocal_approx_fast(out=rs, in_=sg)
                    # --- normalize + prelu ---
                    t = tpool.tile([128, 2 * CH], BF, tag="t",
                                   name=f"t{i}g{g}")
                    if even:
                        nc.vector.tensor_mul(t, cur[g], rs)
                    else:
                        nc.vector.scalar_tensor_tensor(
                            out=t, in0=zps[g], scalar=bcol[i - 1], in1=rs,
                            op0=OP.add, op1=OP.mult)
                    u = upool.tile([128, 2 * CH], BF, tag="u",
                                   name=f"u{i}g{g}")
                    nc.scalar.activation(out=u, in_=t, func=AF.Prelu,
                                         bias=ecol[i], scale=gcol[i],
                                         alpha=acol[i])
                    if not even:
                        u_keep[g][i // 2] = u
                    # --- dense matmul ---
                    zp = zp_pool.tile([128, 2 * CH], FP, tag="zp",
                                      name=f"zp{i + 1}g{g}")
                    for j in range(2):
                        nc.tensor.matmul(
                            out=zp[:, j * CH:(j + 1) * CH],
                            lhsT=wd[:, i * 128:(i + 1) * 128],
                            rhs=u[:, j * CH:(j + 1) * CH],
                            start=True, stop=True)
                    new_zps.append(zp)
                zps = new_zps
                if not even:             # block boundary: materialize Z
                    nxt = []
                    for g in range(NG):
                        zb = anchors.tile([128, 2 * CH], BF, tag="anc",
                                          name=f"zb{i}g{g}")
                        nc.vector.scalar_tensor_tensor(
                            out=zb, in0=zps[g], scalar=bcol[i],
                            in1=anchor[g], op0=OP.add, op1=OP.add)
                        nxt.append(zb)
                    anchor = nxt
                    cur = list(anchor)

            # -- output stage: accumulate mean-track + w_out in PSUM --
            for g in range(NG):
                op_ps = sp_pool.tile([4, 2 * CH], FP, tag="va",
                                     name=f"op{g}",
                                     padded_shape=[128, 2 * CH])
                for j in range(2):
                    k = 2 * g + j
                    sl = slice(j * CH, (j + 1) * CH)
                    nc.tensor.matmul(out=op_ps[:, sl], lhsT=mwin,
                                     rhs=fch(k), start=True, stop=False)
                    for li, l in enumerate((1, 3, 5)):
                        nc.tensor.matmul(
                            out=op_ps[:, sl],
                            lhsT=mw[:, 4 * li: 4 * li + 4],
                            rhs=u_keep[g][li][:, sl],
                            start=False, stop=False)
                    nc.tensor.matmul(out=op_ps[:, sl], lhsT=wout,
                                     rhs=anchor[g][:, sl],
                                     start=False, stop=True)
                ot = opool.tile([4, 2 * CH], FP, tag="ot", name=f"ot{g}")
                nc.vector.tensor_copy(ot, op_ps)
                # planar store: rows (A-re, A-im, B-re, B-im)
                t0 = t0s + 2 * g * CH
                dst = bass.AP(tensor=out.tensor,
                              offset=rp * 2 * TPR + t0,
                              ap=[[4 * TPR, 2], [TPR, 2], [1, 2 * CH]])
                nc.sync.dma_start(out=dst, in_=ot)
    ctx.close()


def _host_pack(inputs):
    """Build the shared (replicated) packed-weight arrays (bf16-ready)."""
    w_in = np.asarray(inputs["w_in"], np.float64)        # [8, 64]
    dense_w = np.asarray(inputs["dense_w"], np.float64)  # [6, 64, 64]
    w_out = np.asarray(inputs["w_out"], np.float64)      # [64, 2]
    ln_gamma = np.asarray(inputs["ln_gamma"], np.float32)
    ln_beta = np.asarray(inputs["ln_beta"], np.float32)
    alpha = np.asarray(inputs["alpha"], np.float32)
    b_in = np.asarray(inputs["b_in"], np.float64)
    dense_b = np.asarray(inputs["dense_b"], np.float64)

    P = np.eye(F) - np.ones((F, F)) / F                  # centering
    s_out = w_out.sum(axis=0)                            # [2] colsums

    w_in_f = w_in @ P
    b_in_f = b_in @ P
    dense_w_f = np.stack([dense_w[l] @ P for l in range(NL)])
    dense_b_f = np.stack([dense_b[l] @ P for l in range(NL)])

    def bd(a, n=2):
        """block-diag replicate [r, c] -> [n*r, n*c]"""
        r, c = a.shape
        o = np.zeros((n * r, n * c), np.float32)
        for q in range(n):
            o[q * r:(q + 1) * r, q * c:(q + 1) * c] = a
        return o

    wd = np.zeros((128, NL * 128), np.float32)
    for l in range(NL):
        wd[:, l * 128:(l + 1) * 128] = bd(dense_w_f[l])
    win = bd(w_in_f)                                     # [16, 128]
    wout = np.zeros((128, 4), np.float32)
    wout[0:64, 0:2] = w_out
    wout[64:128, 2:4] = w_out
    onesd = bd(np.full((F, F), 1.0 / F))

    # mean-track lhsT: cols (A-re, A-im, B-re, B-im), scaled by s_out
    def mtrack(wbar):
        r = len(wbar)
        o = np.zeros((2 * r, 4), np.float32)
        o[0:r, 0] = wbar * s_out[0]
        o[0:r, 1] = wbar * s_out[1]
        o[r:2 * r, 2] = wbar * s_out[0]
        o[r:2 * r, 3] = wbar * s_out[1]
        return o

    mwin = mtrack(w_in.mean(axis=1))                     # [16, 4]
    mw = np.zeros((128, 12), np.float32)
    for li, l in enumerate((1, 3, 5)):
        mw[:, 4 * li: 4 * li + 4] = mtrack(dense_w[l].mean(axis=1))

    percol = np.zeros((128, 26), np.float32)
    percol[:, 0] = np.tile(b_in_f, 2)
    for l in range(NL):
        percol[:, 1 + l] = np.tile(dense_b_f[l], 2)
        percol[:, 7 + l] = np.tile(ln_gamma[l], 2)
        percol[:, 13 + l] = np.tile(ln_beta[l], 2)
        percol[:, 19 + l] = np.tile(alpha[l], 2)

    # host-side constant correction: (mean of each residual-branch bias)*s
    m_const = (b_in.mean() + dense_b[1].mean() + dense_b[3].mean()
               + dense_b[5].mean())
    out_bias = np.asarray(inputs["b_out"], np.float64) + m_const * s_out

    return dict(wd=wd, win=win, wout=wout, mw=mw, mwin=mwin, onesd=onesd,
                percol=percol), out_bias.astype(np.float32)


def build_program(tokens_per_row):
    nc = bacc.Bacc("TRN2")
    ins = {}
    shapes = dict(wd=(128, NL * 128), win=(16, 128), wout=(128, 4),
                  mw=(128, 12), mwin=(16, 4), onesd=(128, 128))
    for name, shp in shapes.items():
        ins[name] = nc.dram_tensor(name, list(shp), BF,
                                   kind="ExternalInput").ap()
    ins["percol"] = nc.dram_tensor("percol", [128, 26], FP,
                                   kind="ExternalInput").ap()
    ins["xr"] = nc.dram_tensor("xr", [4, tokens_per_row], FP,
                               kind="ExternalInput").ap()
    ins["xi"] = nc.dram_tensor("xi", [4, tokens_per_row], FP,
                               kind="ExternalInput").ap()
    outs = {"out": nc.dram_tensor("out", [4, 2, tokens_per_row],
                                  FP, kind="ExternalOutput").ap()}
    with tile.TileContext(nc) as tc:
        build_kernel(tc, outs, ins, tokens_per_row)
    nc.compile()
    return nc


def _to_bf16(a):
    """Round fp32 ndarray to bf16 bit pattern (ml_dtypes if available)."""
    import ml_dtypes
    return a.astype(ml_dtypes.bfloat16)


def _run(inputs, trace=False):
    from concourse.bass_utils import run_bass_kernel_spmd

    x_real = np.ascontiguousarray(np.asarray(inputs["x_real"], np.float32))
    x_imag = np.ascontiguousarray(np.asarray(inputs["x_imag"], np.float32))
    B, N = x_real.shape
    n_cores = 8
    rows_per_core = B // n_cores

    shared, out_bias = _host_pack(inputs)
    shared = {k: (_to_bf16(v) if k != "percol" else v)
              for k, v in shared.items()}
    nc = build_program(N)

    in_maps = []
    for c in range(n_cores):
        m = dict(shared)
        m["xr"] = np.ascontiguousarray(
            x_real[c * rows_per_core:(c + 1) * rows_per_core])
        m["xi"] = np.ascontiguousarray(
            x_imag[c * rows_per_core:(c + 1) * rows_per_core])
        in_maps.append(m)

    res = run_bass_kernel_spmd(nc, in_maps, core_ids=list(range(n_cores)),
                               trace=trace)
    outs_np = [r["out"] for r in res.results]       # each [4, 2, N]
    full = np.concatenate(outs_np, axis=0)          # [B, 2, N]
    re = full[:, 0, :] + out_bias[0] + x_real
    im = full[:, 1, :] + out_bias[1] + x_imag
    return (re + 1j * im).astype(np.complex64), res


def kernel(**inputs):
    return _run(inputs, trace=False)[0]
